# revision 1
# baseline (speedup 1.0000x reference)
import sys

for _p in ("/opt/trn_rl_repo",):
    if _p not in sys.path:
        sys.path.insert(0, _p)

import numpy as np

import concourse.bass as bass
import concourse.bacc as bacc
import concourse.mybir as mybir
from concourse.tile import TileContext
from concourse.bass_utils import run_bass_kernel_spmd

F32 = mybir.dt.float32
BF16 = mybir.dt.bfloat16
GE = mybir.AluOpType.is_ge
EQ = mybir.AluOpType.is_equal

B, N, C, H, W = 4, 4, 256, 100, 152
HH = 50                 # output rows per core (H split in halves)
WP = W + 2              # padded pitch
RB = 5                  # output rows per block
NBLK = HH // RB
SRC_LEN = (RB + 2) * WP          # source elements per block
REG = RB * WP                    # output-region elements per block (770)
BASE = WP + 1                    # offset of output (0,0) in the block source
XLEN = (HH + 2) * WP + 4         # 8012, incl. 4-elem tail slack
PBLK = SRC_LEN + 4               # block tile width (over-read slack)
YLEN = HH * WP                   # 7700
INV9C = 1.0 / (9.0 * C)

SUBS = [(0, 512), (512, REG - 512)]
SUBS_SRC = [(0, 512), (512, 512), (1024, SRC_LEN - 1024)]
SHIFTS = [(di - 1) * WP + (dj - 1) for di in range(3) for dj in range(3)]

# rows in the packed flat staging tile
R_SFLAT, R_SIM = 0, 1

_NC_CACHE = {}


def build_nc():
    nc = bacc.Bacc(trn_type="TRN2")
    x1_h = nc.dram_tensor("x1", [C, XLEN], F32, kind="ExternalInput")
    x2_h = nc.dram_tensor("x2", [N, C, XLEN], F32, kind="ExternalInput")
    wft_h = nc.dram_tensor("wft", [C, C], F32, kind="ExternalInput")  # w_fuse.T [c,o]
    bf_h = nc.dram_tensor("bf", [C, 1], F32, kind="ExternalInput")
    y_h = nc.dram_tensor("y", [C, YLEN], F32, kind="ExternalOutput")

    with TileContext(nc) as tc:
        with (
            tc.tile_pool(name="const", bufs=1) as cpool,
            tc.tile_pool(name="pin1", bufs=2) as p1pool,
            tc.tile_pool(name="pin2", bufs=2) as p2pool,
            tc.tile_pool(name="small", bufs=1) as spool,
            tc.tile_pool(name="flat", bufs=1) as fpool,
            tc.tile_pool(name="arep", bufs=1) as apool,
            tc.tile_pool(name="sig", bufs=1) as sigpool,
            tc.tile_pool(name="scr", bufs=3) as scrpool,
            tc.tile_pool(name="wrep", bufs=1) as wpool,
            tc.tile_pool(name="fus", bufs=1) as fuspool,
            tc.tile_pool(name="ftmp", bufs=1) as ftmppool,
            tc.tile_pool(name="yo", bufs=1) as ypool,
            tc.tile_pool(name="psA", bufs=1, space="PSUM") as psA,
            tc.tile_pool(name="psB", bufs=2, space="PSUM") as psB,
        ):
            ones_col = cpool.tile([128, 1], F32, tag="ones_col")
            nc.vector.memset(ones_col[:], 1.0)
            ones_col_bf = cpool.tile([128, 1], BF16, tag="ones_col_bf")
            nc.vector.memset(ones_col_bf[:], 1.0)
            ones_row = cpool.tile([1, 128], F32, tag="ones_row")
            nc.vector.memset(ones_row[:], 1.0)
            wft = {}
            for cc in range(2):
                for oc in range(2):
                    t = cpool.tile([128, 128], F32, tag=f"wft{cc}{oc}")
                    nc.sync.dma_start(
                        out=t[:],
                        in_=wft_h[cc * 128:(cc + 1) * 128, oc * 128:(oc + 1) * 128],
                    )
                    wft[(cc, oc)] = t
            bft = {}
            for oc in range(2):
                t = cpool.tile([128, 1], F32, tag=f"bf{oc}")
                nc.sync.dma_start(out=t[:], in_=bf_h[oc * 128:(oc + 1) * 128, :])
                bft[oc] = t

            def build_rep(rowtile, pool, tag, nbufs=1):
                ps = psB.tile([128, 1024], F32, tag="psB")
                for (so, sl) in SUBS:
                    nc.tensor.matmul(
                        out=ps[:, so:so + sl],
                        lhsT=ones_row[:],
                        rhs=rowtile[0:1, so:so + sl],
                        start=True,
                        stop=True,
                    )
                rep = pool.tile([128, REG], F32, tag=tag, bufs=nbufs)
                nc.scalar.copy(out=rep[:], in_=ps[:, 0:REG])
                return rep

            for blk in range(NBLK):
                off = blk * REG
                # ---- block loads (src region with halo rows)
                p1 = []
                for cc in range(2):
                    t = p1pool.tile([128, PBLK], F32, tag=f"p1_{cc}")
                    nc.sync.dma_start(
                        out=t[:, 0:PBLK],
                        in_=x1_h[cc * 128:(cc + 1) * 128, off:off + PBLK],
                    )
                    p1.append(t)
                p2 = []
                for n in range(N):
                    row = []
                    for cc in range(2):
                        t = p2pool.tile([128, PBLK], F32, tag=f"p2_{n}_{cc}")
                        nc.sync.dma_start(
                            out=t[:, 0:PBLK],
                            in_=x2_h[n, cc * 128:(cc + 1) * 128, off:off + PBLK],
                        )
                        row.append(t)
                    p2.append(row)

                sflat = fpool.tile([1, 1088], F32, tag="sflat")
                simflat = fpool.tile([1, 772], F32, tag="simflat")
                aflat = [fpool.tile([1, REG], F32, tag=f"aflat{t}", name=f"aflat{t}") for t in range(5)]
                wflat = [fpool.tile([1, REG], F32, tag=f"wflat{n}", name=f"wflat{n}") for n in range(N)]

                # ---- channel sums -> box -> avg (flat rows R_AVG0+t)
                for t5 in range(5):
                    src = p1 if t5 == 0 else p2[t5 - 1]
                    ps = psA.tile([1, 2048], F32, tag="psA")
                    for cc in range(2):
                        for (so, sl) in SUBS_SRC:
                            nc.tensor.matmul(
                                out=ps[0:1, so:so + sl],
                                lhsT=ones_col[:],
                                rhs=src[cc][:, so:so + sl],
                                start=(cc == 0),
                                stop=(cc == 1),
                            )
                    nc.scalar.copy(
                        out=sflat[0:1, 0:SRC_LEN], in_=ps[0:1, 0:SRC_LEN]
                    )
                    sA = spool.tile([RB, WP], F32, tag="sA")
                    sB = spool.tile([RB, WP], F32, tag="sB")
                    sC = spool.tile([RB, WP], F32, tag="sC")
                    nc.sync.dma_start(out=sA[:], in_=sflat[0:1, 0:RB * WP])
                    nc.sync.dma_start(out=sB[:], in_=sflat[0:1, WP:WP + RB * WP])
                    nc.sync.dma_start(
                        out=sC[:], in_=sflat[0:1, 2 * WP:2 * WP + RB * WP]
                    )
                    tv = spool.tile([RB, WP], F32, tag="tv")
                    nc.vector.tensor_add(out=tv[:], in0=sA[:], in1=sB[:])
                    nc.vector.tensor_add(out=tv[:], in0=tv[:], in1=sC[:])
                    av = spool.tile([RB, WP], F32, tag="av")
                    nc.vector.tensor_add(
                        out=av[:, 0:W], in0=tv[:, 0:W], in1=tv[:, 1:W + 1]
                    )
                    nc.vector.tensor_add(
                        out=av[:, 0:W], in0=av[:, 0:W], in1=tv[:, 2:W + 2]
                    )
                    nc.vector.tensor_scalar(
                        out=av[:, 0:W], in0=av[:, 0:W], scalar1=INV9C,
                        scalar2=None, op0=mybir.AluOpType.mult,
                    )
                    nc.vector.memset(av[:, W:WP], 0.0)
                    nc.sync.dma_start(out=aflat[t5][0:1, 0:REG], in_=av[:])

                a1rep = build_rep(aflat[0], apool, "a1rep")

                # ---- sigma1 bits for all 9 shifts
                sig1 = {}
                for k, ok in enumerate(SHIFTS):
                    for cc in range(2):
                        sg = sigpool.tile([128, REG], BF16, tag=f"sig{k}_{cc}")
                        nc.vector.tensor_tensor(
                            out=sg[:],
                            in0=p1[cc][:, BASE + ok:BASE + ok + REG],
                            in1=a1rep[:],
                            op=GE,
                        )
                        sig1[(k, cc)] = sg

                # ---- per-n similarity
                sim2d = []
                for n in range(N):
                    a2rep = build_rep(aflat[1 + n], apool, "a2rep", nbufs=2)
                    ps = psA.tile([1, 2048], F32, tag="psA")
                    for k, ok in enumerate(SHIFTS):
                        for cc in range(2):
                            sg2 = scrpool.tile([128, REG], BF16, tag="sg2")
                            nc.vector.tensor_tensor(
                                out=sg2[:],
                                in0=p2[n][cc][:, BASE + ok:BASE + ok + REG],
                                in1=a2rep[:],
                                op=GE,
                            )
                            xn = scrpool.tile([128, REG], BF16, tag="xn")
                            nc.vector.tensor_tensor(
                                out=xn[:], in0=sig1[(k, cc)][:], in1=sg2[:], op=EQ
                            )
                            for (so, sl) in SUBS:
                                nc.tensor.matmul(
                                    out=ps[0:1, so:so + sl],
                                    lhsT=ones_col_bf[:],
                                    rhs=xn[:, so:so + sl],
                                    start=(k == 0 and cc == 0),
                                    stop=(k == 8 and cc == 1),
                                )
                    nc.scalar.copy(
                        out=simflat[0:1, 0:REG], in_=ps[0:1, 0:REG]
                    )
                    s2 = spool.tile([RB, WP], F32, tag=f"sim2d{n}")
                    nc.sync.dma_start(out=s2[:], in_=simflat[0:1, 0:REG])
                    sim2d.append(s2)

                # ---- softmax over n on [RB, WP] tiles
                mx = spool.tile([RB, WP], F32, tag="mx")
                nc.vector.tensor_tensor(
                    out=mx[:], in0=sim2d[0][:], in1=sim2d[1][:],
                    op=mybir.AluOpType.max,
                )
                nc.vector.tensor_tensor(
                    out=mx[:], in0=mx[:], in1=sim2d[2][:], op=mybir.AluOpType.max
                )
                nc.vector.tensor_tensor(
                    out=mx[:], in0=mx[:], in1=sim2d[3][:], op=mybir.AluOpType.max
                )
                es = []
                for n in range(N):
                    d = spool.tile([RB, WP], F32, tag=f"ed{n}")
                    nc.vector.tensor_tensor(
                        out=d[:], in0=sim2d[n][:], in1=mx[:],
                        op=mybir.AluOpType.subtract,
                    )
                    nc.scalar.activation(
                        out=d[:], in_=d[:], func=mybir.ActivationFunctionType.Exp
                    )
                    es.append(d)
                den = spool.tile([RB, WP], F32, tag="den")
                nc.vector.tensor_add(out=den[:], in0=es[0][:], in1=es[1][:])
                nc.vector.tensor_add(out=den[:], in0=den[:], in1=es[2][:])
                nc.vector.tensor_add(out=den[:], in0=den[:], in1=es[3][:])
                rec = spool.tile([RB, WP], F32, tag="rec")
                nc.vector.reciprocal(out=rec[:], in_=den[:])

                wrep = []
                for n in range(N):
                    wv = spool.tile([RB, WP], F32, tag=f"wv{n}")
                    nc.vector.tensor_mul(out=wv[:], in0=es[n][:], in1=rec[:])
                    nc.sync.dma_start(out=wflat[n][0:1, 0:REG], in_=wv[:])
                    wrep.append(build_rep(wflat[n], wpool, f"wrep{n}"))

                # ---- fusion: fused = p1 + sum_n w_n * p2_n ; then 1x1 conv
                fused = []
                for cc in range(2):
                    fu = fuspool.tile([128, REG], F32, tag=f"fu{cc}")
                    nc.vector.tensor_copy(
                        out=fu[:], in_=p1[cc][:, BASE:BASE + REG]
                    )
                    for n in range(N):
                        tmp = ftmppool.tile([128, REG], F32, tag="ftmp")
                        nc.vector.tensor_mul(
                            out=tmp[:],
                            in0=wrep[n][:],
                            in1=p2[n][cc][:, BASE:BASE + REG],
                        )
                        nc.vector.tensor_add(out=fu[:], in0=fu[:], in1=tmp[:])
                    fused.append(fu)

                for oc in range(2):
                    ps = psB.tile([128, 1024], F32, tag="psB")
                    for cc in range(2):
                        for (so, sl) in SUBS:
                            nc.tensor.matmul(
                                out=ps[:, so:so + sl],
                                lhsT=wft[(cc, oc)][:],
                                rhs=fused[cc][:, so:so + sl],
                                start=(cc == 0),
                                stop=(cc == 1),
                            )
                    yo = ypool.tile([128, REG], F32, tag=f"yo{oc}")
                    nc.scalar.activation(
                        out=yo[:],
                        in_=ps[:, 0:REG],
                        func=mybir.ActivationFunctionType.Identity,
                        bias=bft[oc][:],
                        scale=1.0,
                    )
                    nc.sync.dma_start(
                        out=y_h[oc * 128:(oc + 1) * 128, off:off + REG], in_=yo[:]
                    )
    nc.compile()
    return nc


def get_nc():
    if "nc" not in _NC_CACHE:
        _NC_CACHE["nc"] = build_nc()
    return _NC_CACHE["nc"]


def shard_inputs(features, nearby_features, w_fuse, b_fuse):
    features = np.asarray(features, np.float32)
    nearby_features = np.asarray(nearby_features, np.float32)
    wft = np.ascontiguousarray(np.asarray(w_fuse, np.float32).T)
    bf = np.ascontiguousarray(np.asarray(b_fuse, np.float32).reshape(C, 1))
    cidx = np.clip(np.arange(-1, W + 1), 0, W - 1)
    in_maps = []
    for b in range(B):
        for half in range(2):
            h0 = half * HH
            ridx = np.clip(np.arange(h0 - 1, h0 + HH + 1), 0, H - 1)
            x1p = features[b][:, ridx][:, :, cidx].reshape(C, -1)
            x1 = np.zeros((C, XLEN), np.float32)
            x1[:, : x1p.shape[1]] = x1p
            x2p = nearby_features[b][:, :, ridx][:, :, :, cidx].reshape(N, C, -1)
            x2 = np.zeros((N, C, XLEN), np.float32)
            x2[:, :, : x2p.shape[2]] = x2p
            in_maps.append(
                {
                    "x1": np.ascontiguousarray(x1),
                    "x2": np.ascontiguousarray(x2),
                    "wft": wft,
                    "bf": bf,
                }
            )
    return in_maps


def gather_output(results):
    out = np.empty((B, C, H, W), np.float32)
    for i, r in enumerate(results):
        b, half = i // 2, i % 2
        y = np.asarray(r["y"]).reshape(C, HH, WP)[:, :, :W]
        out[b, :, half * HH:(half + 1) * HH, :] = y
    return out


def kernel(features, nearby_features, w_fuse, b_fuse, _trace=False, _trace_kwargs=None):
    in_maps = shard_inputs(features, nearby_features, w_fuse, b_fuse)
    nc = get_nc()
    kw = {}
    if _trace:
        kw = dict(trace=True, **(_trace_kwargs or {}))
    res = run_bass_kernel_spmd(nc, in_maps, core_ids=list(range(8)), **kw)
    out = gather_output(res.results)
    kernel._last_result = res
    return out

